# revision 10
# baseline (speedup 1.0000x reference)
"""Bipartite GNN message-passing layer on 8 Trainium2 NeuronCores.

Strategy: shard target nodes across the 8 cores; partition edges by target
so the scatter-mean is local; replicate source features + weights.

v2 (vs baseline):
  - host-side target->(core,tile,slot) balancing so every (tile, src-half)
    edge bucket fits its block budget with ~0.4% padding (was ~30%);
  - flexible A/B source split (A window [0,32768), B window [17232,50000),
    overlap assigned per (core,tile) to equalize the halves) -> 6 blocks
    per tile, 294 per direction (was ~390);
  - one dma_gather per (supertile, group) (28 calls total, was ~200) to
    amortize the ~1us SWDGE fixed overhead;
  - bf16 residual input, output, weights/bias and dense matmul (4x PE
    rate vs f32r, and half the HBM bytes for x/out);
  - LayerNorm stats + final scale fused per-supertile into the main
    pipeline (the per-direction epilogue used to idle the DMA queues
    for ~55us per direction).

Per core, per direction, per 128-target tile:
  fp8 one-hot scatter matmul (segment sum on the PE) -> mean+residual on
  DVE -> PE transpose -> bf16 dense matmul + bias -> relu on ACT with
  row-sum accumulation -> squared-sum on DVE -> per-supertile LN stats
  -> per-tile scale+shift on ACT -> bf16 store.
"""

import os
import sys

if "/opt/trn_rl_repo" not in sys.path:
    sys.path.insert(0, "/opt/trn_rl_repo")

from contextlib import ExitStack

import ml_dtypes
import numpy as np

D = 256
NNODE = 50000
N_CORES = 8
TPC = NNODE // N_CORES  # targets per core
TILE = 128
NT = (TPC + TILE - 1) // TILE  # 49 target tiles per core
NTP = NT * TILE  # padded target rows per core (6272)
FLEXHI = 32768  # A window: src in [0, 32768)
FLEXLO = NNODE - 32768  # 17232; B window: src in [17232, 50000)
SUPER = 7  # tiles per supertile (7 x 7 = 49)
NSUP = (NT + SUPER - 1) // SUPER
GCHUNK = 8  # gather call size in 128-edge blocks (1024 idx = max single_packet)
EPI_DELAY = 2  # tiles between a supertile's last C and its LN epilogue
EPS = 1e-5

F8 = ml_dtypes.float8_e4m3
BF16 = ml_dtypes.bfloat16

# test-only hooks (harness leaves these off)
_TRACE = bool(os.environ.get("BGK_TRACE"))
last_result = None

_prog_cache = {}


def _wrap_idx(idx):
    """dma_gather index layout: edge i -> [i % 16, i // 16], replicated
    across the 8 Q7-core partition groups."""
    assert len(idx) % 16 == 0
    w = idx.reshape(-1, 16).T.astype(np.int16)  # [16, n/16]
    return np.tile(w, (8, 1))  # [128, n/16]


def _balance_targets(a_deg, b_deg, deg_t):
    """Assign each target node to a (core, tile, slot) so that per-bin
    A-only/B-only/total edge loads are level. Returns (bin, slot) per
    target."""
    NBINS = N_CORES * NT
    cap = np.full(NBINS, TILE, np.int64)
    cap[np.arange(N_CORES) * NT + NT - 1] = TPC - (NT - 1) * TILE
    order = np.argsort(-deg_t, kind="stable")
    sumA = np.zeros(NBINS)
    sumB = np.zeros(NBINS)
    sumT = np.zeros(NBINS)
    nslots = np.zeros(NBINS, np.int64)
    assign = np.empty(NNODE, np.int64)
    slot = np.empty(NNODE, np.int64)
    pos = 0
    while pos < NNODE:
        ob = np.nonzero(nslots < cap)[0]
        k = min(len(ob), NNODE - pos)
        batch = order[pos : pos + k]
        load = np.maximum(
            np.maximum(sumA[ob] / 384.0, sumB[ob] / 384.0), sumT[ob] / 768.0
        )
        chosen = ob[np.argsort(load, kind="stable")][:k]
        assign[batch] = chosen
        slot[batch] = nslots[chosen]
        sumA[chosen] += a_deg[batch]
        sumB[chosen] += b_deg[batch]
        sumT[chosen] += deg_t[batch]
        nslots[chosen] += 1
        pos += k
    return assign, slot


def _prep_direction(src, tgt):
    """Host-side balance/shard/sort/pad. Returns (nblk, blk_off, totblk,
    per-core dict of arrays, per-core target permutation)."""
    deg_t = np.bincount(tgt, minlength=NNODE).astype(np.int64)
    recip_full = (1.0 / np.maximum(deg_t, 1.0)).astype(np.float32)
    a_deg = np.bincount(tgt[src < FLEXLO], minlength=NNODE)
    b_deg = np.bincount(tgt[src >= FLEXHI], minlength=NNODE)

    assign, slot = _balance_targets(a_deg, b_deg, deg_t)

    # per-(core,tile) loads
    aa = np.zeros(N_CORES * NT, np.int64)
    bb = np.zeros(N_CORES * NT, np.int64)
    tt_ld = np.zeros(N_CORES * NT, np.int64)
    np.add.at(aa, assign, a_deg)
    np.add.at(bb, assign, b_deg)
    np.add.at(tt_ld, assign, deg_t)
    aa = aa.reshape(N_CORES, NT)
    bb = bb.reshape(N_CORES, NT)
    tt_ld = tt_ld.reshape(N_CORES, NT)
    ff = tt_ld - aa - bb

    # per-tile block budget (kA, kB), shared by all cores
    nblk = np.zeros((NT, 2), np.int64)
    for ti in range(NT):
        done = False
        for tot in range(max(1, int(np.ceil(tt_ld[:, ti].max() / 128))), 24):
            for kA in range(0, tot + 1):
                kB = tot - kA
                if (
                    (aa[:, ti] <= kA * 128).all()
                    and (bb[:, ti] <= kB * 128).all()
                    and (tt_ld[:, ti] <= tot * 128).all()
                ):
                    nblk[ti] = (kA, kB)
                    done = True
                    break
            if done:
                break
        assert done, f"tile {ti} infeasible"
    totblk = int(nblk.sum())
    blk_off = np.concatenate(
        [np.zeros((2, 1), np.int64), np.cumsum(nblk.T, axis=1)], axis=1
    )  # [2, NT+1]
    totA = int(blk_off[0, NT])

    # per-(core,tile) flex split: first fA flex edges -> group A
    kA128 = nblk[None, :, 0] * 128
    kB128 = nblk[None, :, 1] * 128
    fA = np.clip(ff - (kB128 - bb), 0, kA128 - aa)
    assert (aa + fA <= kA128).all() and (bb + ff - fA <= kB128).all()

    # per-edge core/tile/slot/group
    e_bin = assign[tgt]
    e_slot = slot[tgt]
    e_src = src
    grp = np.where(e_src < FLEXLO, 0, np.where(e_src >= FLEXHI, 1, 2))
    flex = np.nonzero(grp == 2)[0]
    fo = flex[np.argsort(e_bin[flex], kind="stable")]
    fb = e_bin[fo]
    starts = np.searchsorted(fb, np.arange(N_CORES * NT))
    rank = np.arange(len(fo)) - starts[fb]
    to_a = rank < fA.reshape(-1)[fb]
    grp2 = grp.copy()
    grp2[fo[to_a]] = 0
    grp2[fo[~to_a]] = 1

    cores = []
    for c in range(N_CORES):
        m = (e_bin // NT) == c
        ti_e = (e_bin[m] % NT).astype(np.int64)
        sl_e = e_slot[m]
        s_e = e_src[m]
        g_e = grp2[m]
        key = (ti_e * 2 + g_e).astype(np.int64)
        o = np.argsort(key, kind="stable")
        ti_e, sl_e, s_e, g_e, key = ti_e[o], sl_e[o], s_e[o], g_e[o], key[o]
        seg = np.searchsorted(key, np.arange(NT * 2 + 1))

        idx_cat = [[], []]
        S = np.zeros((128, totblk * 128), np.uint8)
        for ti in range(NT):
            for g in range(2):
                lo, hi = seg[ti * 2 + g], seg[ti * 2 + g + 1]
                n = int(nblk[ti, g]) * 128
                ne = hi - lo
                assert ne <= n
                idx = np.zeros(n, np.int64)
                sv = s_e[lo:hi] if g == 0 else s_e[lo:hi] - FLEXLO
                idx[:ne] = sv
                idx_cat[g].append(idx)
                if n:
                    j = np.arange(ne)
                    base = int(blk_off[g, ti]) * 128
                    if g == 1:
                        base += totA * 128
                    S[j % 128, base + (j // 128) * 128 + sl_e[lo:hi]] = 0x38
        idx_g = [_wrap_idx(np.concatenate(idx_cat[g])) for g in range(2)]

        perm = np.full(NTP, -1, np.int64)
        tmask = (assign // NT) == c
        t_ids = np.nonzero(tmask)[0]
        rows = (assign[t_ids] % NT) * TILE + slot[t_ids]
        perm[rows] = t_ids
        recip = np.zeros(NTP, np.float32)
        recip[rows] = recip_full[t_ids]
        cores.append(
            {
                "idxA": idx_g[0],
                "idxB": idx_g[1],
                "S": S.view(F8),
                "recip": recip.reshape(NT, 128).T.copy(),  # [128, NT]
                "perm": perm,
            }
        )
    return nblk, blk_off, totblk, cores


def _build_program(meta_u, meta_i, apply_gamma_beta):
    import concourse.bass as bass
    import concourse.tile as tile
    from concourse import bacc, mybir

    f32 = mybir.dt.float32
    bf16 = mybir.dt.bfloat16
    f8 = mybir.dt.float8e4
    i16 = mybir.dt.int16
    Alu = mybir.AluOpType
    Act = mybir.ActivationFunctionType

    nc = bacc.Bacc("TRN2", target_bir_lowering=False, debug=False,
                   num_devices=N_CORES, num_swdge_queues=4)

    def din(name, shape, dt):
        return nc.dram_tensor(name, shape, dt, kind="ExternalInput").ap()

    dirs = []
    for d, (nblk, blk_off, totblk) in (("u", meta_u), ("i", meta_i)):
        totA, totB = int(blk_off[0, NT]), int(blk_off[1, NT])
        dirs.append(
            {
                "name": d,
                "nblk": nblk,
                "blk_off": blk_off,
                "totblk": totblk,
                "totg": (totA, totB),
                "src16": din(f"src16_{d}", [NNODE, D], bf16),
                "x": din(f"x_{d}", [NTP, D], bf16),
                "W": din(f"W_{d}", [2, 128, D], bf16),
                "bias": din(f"bias_{d}", [1, D], bf16),
                "idx": [
                    din(f"idx0_{d}", [128, totA * 8], i16),
                    din(f"idx1_{d}", [128, totB * 8], i16),
                ],
                "S": din(f"S_{d}", [128, totblk * 128], f8),
                "recip": din(f"recip_{d}", [128, NT], f32),
                "out": nc.dram_tensor(
                    f"out_{d}", [NTP, D], bf16, kind="ExternalOutput"
                ).ap(),
            }
        )
    ident_d = din("ident", [128, 128], bf16)
    ones_d = din("ones", [1, 128], bf16)
    if apply_gamma_beta:
        gamma_d = din("gamma_rep", [128, D], f32)
        beta_d = din("beta_rep", [128, D], f32)

    qctr = [0]  # SWDGE queue round-robin

    with tile.TileContext(nc) as tc, ExitStack() as ctx:
        consts = ctx.enter_context(tc.tile_pool(name="consts", bufs=1))
        msgs_p = ctx.enter_context(tc.tile_pool(name="msgs", bufs=3))
        s_p = ctx.enter_context(tc.tile_pool(name="sp", bufs=3))
        xio_p = ctx.enter_context(tc.tile_pool(name="xio", bufs=2))
        work = ctx.enter_context(tc.tile_pool(name="work", bufs=3))
        psum_a = ctx.enter_context(tc.tile_pool(name="psa", bufs=4, space="PSUM"))
        psum_t = ctx.enter_context(tc.tile_pool(name="pst", bufs=2, space="PSUM"))
        psum_y = ctx.enter_context(tc.tile_pool(name="psy", bufs=2, space="PSUM"))

        ident_t = consts.tile([128, 128], bf16)
        nc.sync.dma_start(ident_t[:], ident_d[:])
        ones_t = consts.tile([1, 128], bf16)
        nc.sync.dma_start(ones_t[:], ones_d[:])
        if apply_gamma_beta:
            gamma_t = consts.tile([128, D], f32)
            nc.sync.dma_start(gamma_t[:], gamma_d[:])
            beta_t = consts.tile([128, D], f32)
            nc.sync.dma_start(beta_t[:], beta_d[:])

        # Hoist both directions' metadata + arenas so direction i's input
        # loads and first gathers stream during direction u's tail compute
        # (a per-direction pool teardown would alias the SBUF and serialize
        # them via WAR deps).
        for dd in dirs:
            d = dd["name"]
            dmeta = ctx.enter_context(tc.tile_pool(name=f"meta_{d}", bufs=1))
            dd["idx_t"] = []
            for g in range(2):
                it = dmeta.tile([128, dd["totg"][g] * 8], i16, name=f"idx{g}_{d}")
                nc.sync.dma_start(it[:], dd["idx"][g][:])
                dd["idx_t"].append(it)
            W_t = dmeta.tile([128, 2, D], bf16, name=f"W_{d}")
            for h in range(2):
                nc.sync.dma_start(W_t[:, h, :], dd["W"][h])
            dd["W_t"] = W_t
            bias_t = dmeta.tile([1, D], bf16, name=f"bias_{d}")
            nc.sync.dma_start(bias_t[:], dd["bias"][:])
            dd["bias_t"] = bias_t
            recip_t = dmeta.tile([128, NT], f32, name=f"recip_{d}")
            nc.sync.dma_start(recip_t[:], dd["recip"][:])
            dd["recip_t"] = recip_t
            dd["yr"] = dmeta.tile([128, NT, D], bf16, name=f"yr_{d}")
            dd["s1"] = dmeta.tile([128, NT], f32, name=f"s1_{d}")
            dd["s2"] = dmeta.tile([128, NT], f32, name=f"s2_{d}")
            dd["rstd"] = dmeta.tile([128, NT], f32, name=f"rstd_{d}")
            dd["shift"] = dmeta.tile([128, NT], f32, name=f"shift_{d}")

        for dd in dirs:
            d = dd["name"]
            nblk, blk_off = dd["nblk"], dd["blk_off"]
            idx_t, W_t, bias_t, recip_t = (
                dd["idx_t"], dd["W_t"], dd["bias_t"], dd["recip_t"]
            )
            yr_all, s1_all, s2_all = dd["yr"], dd["s1"], dd["s2"]
            rstd_all, shift_all = dd["rstd"], dd["shift"]

            # Software-pipelined emission with a 2-tile skew so the PE
            # instruction stream never waits on a same-tile DVE round
            # trip: A(t) scatter-matmuls, B(t-1) mean+residual+transpose,
            # C(t-2) dense+relu+sq. LN stats + final scale + store are
            # emitted EPI_DELAY tiles after a supertile's last C so their
            # semaphore waits are already satisfied when the in-order DVE
            # reaches them (no head-of-line blocking).
            state = {}

            def emit_A(ti, msgs, s_tiles):
                agg = psum_a.tile([128, D], f32, tag="agg",
                                  name=f"agg_{d}_{ti}")
                tot_tile_blocks = int(nblk[ti, 0] + nblk[ti, 1])
                done = 0
                for g in range(2):
                    nb = int(nblk[ti, g])
                    if nb == 0:
                        continue
                    m, b0 = msgs[g]
                    sb = s_tiles[g]
                    lo = int(blk_off[g, ti]) - b0
                    for k in range(nb):
                        nc.tensor.matmul(
                            agg[:],
                            lhsT=sb[:, lo + k, :],
                            rhs=m[:, lo + k, :],
                            start=(done == 0),
                            stop=(done == tot_tile_blocks - 1),
                        )
                        done += 1
                state[ti] = {"agg": agg}

            def emit_B(ti, x_sup, tl):
                st = state[ti]
                xm = work.tile([128, D], bf16, tag="xm", name=f"xm_{d}_{ti}")
                nc.vector.scalar_tensor_tensor(
                    xm[:], st["agg"][:], recip_t[:, ti : ti + 1],
                    x_sup[:, tl, :], Alu.mult, Alu.add,
                )
                tr = psum_t.tile([128, 2, 128], bf16, tag="tr",
                                 name=f"tr_{d}_{ti}")
                for h in range(2):
                    nc.tensor.transpose(
                        tr[:, h, :], xm[:, h * 128 : (h + 1) * 128],
                        ident_t[:],
                    )
                xmT = work.tile([128, 2, 128], bf16, tag="xmT",
                                name=f"xmT_{d}_{ti}")
                nc.vector.tensor_copy(xmT[:], tr[:])
                st["xmT"] = xmT

            def emit_C(ti):
                st = state.pop(ti)
                xmT = st["xmT"]
                y_ps = psum_y.tile([128, D], f32, tag="y",
                                   name=f"y_{d}_{ti}")
                nc.tensor.matmul(y_ps[:], lhsT=ones_t[:], rhs=bias_t[:],
                                 start=True, stop=False)
                for h in range(2):
                    nc.tensor.matmul(
                        y_ps[:], lhsT=xmT[:, h, :], rhs=W_t[:, h, :],
                        start=False, stop=(h == 1),
                    )
                yr = yr_all[:, ti, :]
                nc.scalar.activation(
                    yr, y_ps[:], Act.Relu,
                    accum_out=s1_all[:, ti : ti + 1],
                )
                sq = work.tile([128, D], bf16, tag="sq", name=f"sq_{d}_{ti}")
                nc.vector.scalar_tensor_tensor(
                    sq[:], yr, 1.0, yr, Alu.mult, Alu.mult,
                    accum_out=s2_all[:, ti : ti + 1],
                )

            def emit_epilogue(si):
                t0, t1 = si * SUPER, min((si + 1) * SUPER, NT)
                nts = t1 - t0
                sl = slice(t0, t1)
                # rstd = 1/sqrt(var+eps), var = s2/D - (s1/D)^2
                msq = work.tile([128, SUPER], f32, tag="msq",
                                name=f"msq_{d}_{si}")[:, :nts]
                nc.vector.tensor_tensor(msq, s1_all[:, sl], s1_all[:, sl],
                                        Alu.mult)
                t1v = work.tile([128, SUPER], f32, tag="t1v",
                                name=f"t1v_{d}_{si}")[:, :nts]
                nc.vector.scalar_tensor_tensor(
                    t1v, msq, -1.0 / D, s2_all[:, sl], Alu.mult, Alu.add
                )
                veps = work.tile([128, SUPER], f32, tag="veps",
                                 name=f"veps_{d}_{si}")[:, :nts]
                nc.vector.tensor_scalar(
                    veps, t1v, 1.0 / D, EPS, Alu.mult, Alu.add
                )
                rv = work.tile([128, SUPER], f32, tag="rv",
                               name=f"rv_{d}_{si}")[:, :nts]
                nc.vector.reciprocal(rv, veps)
                nc.scalar.activation(rstd_all[:, sl], rv, Act.Sqrt)
                nc.vector.scalar_tensor_tensor(
                    shift_all[:, sl], s1_all[:, sl], -1.0 / D,
                    rstd_all[:, sl], Alu.mult, Alu.mult,
                )
                out_sup = xio_p.tile([128, SUPER, D], bf16, tag="os",
                                     name=f"os_{d}_{si}")
                for ti in range(t0, t1):
                    tl = ti - t0
                    o = out_sup[:, tl, :]
                    nc.scalar.activation(
                        o, yr_all[:, ti, :], Act.Identity,
                        bias=shift_all[:, ti : ti + 1],
                        scale=rstd_all[:, ti : ti + 1],
                    )
                    if apply_gamma_beta:
                        nc.vector.tensor_tensor(o, o, gamma_t[:], Alu.mult)
                        nc.vector.tensor_tensor(o, o, beta_t[:], Alu.add)
                nc.sync.dma_start(
                    dd["out"][t0 * TILE : t1 * TILE].rearrange(
                        "(t p) c -> p t c", p=128
                    ),
                    out_sup[:, :nts, :],
                )

            next_epi = [0]

            def after_C(ti, flush=False):
                while next_epi[0] < NSUP:
                    si = next_epi[0]
                    t_last = min((si + 1) * SUPER, NT) - 1
                    if not flush and ti < t_last + EPI_DELAY:
                        break
                    emit_epilogue(si)
                    next_epi[0] += 1
                    if not flush:
                        break

            xsup_of = {}
            pend = []  # tiles awaiting B (then C)
            for si in range(NSUP):
                t0, t1 = si * SUPER, min((si + 1) * SUPER, NT)
                nts = t1 - t0
                msgs, s_tiles = [], []
                for g in range(2):
                    b0, b1 = int(blk_off[g, t0]), int(blk_off[g, t1])
                    nb = b1 - b0
                    if nb == 0:
                        msgs.append((None, b0))
                        s_tiles.append(None)
                        continue
                    m = msgs_p.tile([128, nb, D], bf16, tag=f"msgs{g}",
                                    name=f"msgs{g}_{d}_{si}")
                    src_view = (
                        dd["src16"][0:FLEXHI] if g == 0
                        else dd["src16"][FLEXLO:NNODE]
                    )
                    for c0 in range(0, nb, GCHUNK):
                        c1 = min(c0 + GCHUNK, nb)
                        nc.gpsimd.dma_gather(
                            m[:, c0:c1, :],
                            src_view,
                            idx_t[g][:, (b0 + c0) * 8 : (b0 + c1) * 8],
                            num_idxs=(c1 - c0) * 128,
                            num_idxs_reg=(c1 - c0) * 128,
                            elem_size=D,
                            single_packet=True,
                            queue_num=qctr[0] % 4,
                        )
                        qctr[0] += 1
                    msgs.append((m, b0))
                    # S slab for this supertile+group
                    sb = s_p.tile([128, nb, 128], f8, tag=f"S{g}",
                                  name=f"S{g}_{d}_{si}")
                    soff = b0 + (dd["totg"][0] if g == 1 else 0)
                    nc.sync.dma_start(
                        sb[:],
                        dd["S"][:, soff * 128 : (soff + nb) * 128],
                    )
                    s_tiles.append(sb)

                x_sup = xio_p.tile([128, SUPER, D], bf16, tag="xs",
                                   name=f"xs_{d}_{si}")
                nc.sync.dma_start(
                    x_sup[:, :nts, :],
                    dd["x"][t0 * TILE : t1 * TILE].rearrange(
                        "(t p) c -> p t c", p=128
                    ),
                )
                for ti in range(t0, t1):
                    xsup_of[ti] = (x_sup, ti - t0)
                    emit_A(ti, msgs, s_tiles)
                    pend.append(ti)
                    if len(pend) >= 2:
                        tb = pend[-2]
                        if tb in xsup_of:
                            emit_B(tb, *xsup_of.pop(tb))
                    if len(pend) >= 3:
                        tc_ = pend.pop(0)
                        emit_C(tc_)
                        after_C(tc_)
            # flush
            if len(pend) >= 1 and pend[-1] in xsup_of:
                emit_B(pend[-1], *xsup_of.pop(pend[-1]))
            for ti in pend:
                emit_C(ti)
                after_C(ti)
            pend.clear()
            after_C(NT - 1, flush=True)

    nc.compile()
    return nc


def kernel(
    user_features,
    item_features,
    user_item_edge_index,
    item_user_edge_index,
    Wu,
    bu,
    Wi,
    bi,
    gamma,
    beta,
):
    from concourse.bass_utils import run_bass_kernel_spmd

    uf = np.asarray(user_features, np.float32)
    itf = np.asarray(item_features, np.float32)
    ui = np.asarray(user_item_edge_index)
    iu = np.asarray(item_user_edge_index)
    Wu = np.asarray(Wu, np.float32)
    Wi = np.asarray(Wi, np.float32)
    bu = np.asarray(bu, np.float32)
    bi = np.asarray(bi, np.float32)
    gamma_np = np.asarray(gamma, np.float32)
    beta_np = np.asarray(beta, np.float32)

    # direction "u": targets are users, sources are items
    nblk_u, off_u, tot_u, cores_u = _prep_direction(
        iu[0].astype(np.int64), iu[1].astype(np.int64)
    )
    # direction "i": targets are items, sources are users
    nblk_i, off_i, tot_i, cores_i = _prep_direction(
        ui[0].astype(np.int64), ui[1].astype(np.int64)
    )

    apply_gb = not (np.all(gamma_np == 1.0) and np.all(beta_np == 0.0))

    key = (nblk_u.tobytes(), nblk_i.tobytes(), apply_gb)
    if key not in _prog_cache:
        _prog_cache[key] = _build_program(
            (nblk_u, off_u, tot_u), (nblk_i, off_i, tot_i), apply_gb
        )
    nc = _prog_cache[key]

    ident = np.eye(128, dtype=np.float32).astype(BF16)
    ones = np.ones((1, 128), BF16)
    src16_u = itf.astype(BF16)  # sources for direction u are items
    src16_i = uf.astype(BF16)
    uf16 = uf.astype(BF16)
    itf16 = itf.astype(BF16)

    def pad_x(x16, perm):
        out = np.zeros((NTP, D), BF16)
        v = perm >= 0
        out[v] = x16[perm[v]]
        return out

    in_maps = []
    for c in range(N_CORES):
        im = {
            "src16_u": src16_u,
            "src16_i": src16_i,
            "x_u": pad_x(uf16, cores_u[c]["perm"]),
            "x_i": pad_x(itf16, cores_i[c]["perm"]),
            "W_u": Wu.astype(BF16).reshape(2, 128, D),
            "W_i": Wi.astype(BF16).reshape(2, 128, D),
            "bias_u": bu.astype(BF16).reshape(1, D),
            "bias_i": bi.astype(BF16).reshape(1, D),
            "recip_u": cores_u[c]["recip"],
            "recip_i": cores_i[c]["recip"],
            "idx0_u": cores_u[c]["idxA"],
            "idx1_u": cores_u[c]["idxB"],
            "idx0_i": cores_i[c]["idxA"],
            "idx1_i": cores_i[c]["idxB"],
            "S_u": cores_u[c]["S"],
            "S_i": cores_i[c]["S"],
            "ident": ident,
            "ones": ones,
        }
        if apply_gb:
            im["gamma_rep"] = np.tile(gamma_np[None, :], (128, 1))
            im["beta_rep"] = np.tile(beta_np[None, :], (128, 1))
        in_maps.append(im)

    res = run_bass_kernel_spmd(nc, in_maps, list(range(N_CORES)), trace=_TRACE)
    global last_result
    last_result = res
    u_new = np.empty((NNODE, D), np.float32)
    i_new = np.empty((NNODE, D), np.float32)
    for c in range(N_CORES):
        for (cores, out, name) in (
            (cores_u, u_new, "out_u"),
            (cores_i, i_new, "out_i"),
        ):
            perm = cores[c]["perm"]
            v = perm >= 0
            out[perm[v]] = res.results[c][name][v].astype(np.float32)
    return (u_new, i_new)
